# revision 26
# baseline (speedup 1.0000x reference)
"""Causal self-attention on 8 Trainium2 NeuronCores — v3 (dependency untangle).

Sharding: 2 heads per core (tensor parallel), host sums the 8 row-parallel
projection partials.  Same dataflow as v2, but the v2 trace (298us traced /
185us untraced) showed 122us of PE idle in 2-14us gaps at q-block
boundaries: the normalize chain (DVE s0-copy -> DVE recip -> GpSimd
partition_broadcast -> DVE mult) serialized with the trimask mult and V1
copies on GpSimd's FIFO, head-of-line blocking the next PV matmul.  Those
gaps also re-throttled HAM (PE at 1.2GHz for ~half the kernel).

v3 changes:
  - PV stationary is [ones(64) | V(64)] (128 cols): PSUM rows 0:64 get the
    softmax sums replicated 64x, rows 64:128 the values.  Normalize is one
    DVE reciprocal_approx_fast on [64, 2, TB] (partition-0 source, reads
    PSUM directly) + one DVE mult per head.  No GpSimd broadcast, no
    [1,N] micro-ops, no s0 copy.
  - The V DMA-transpose writes straight into the [ones|V] tile (strided
    dst), deleting the per-block GpSimd staging copy.
  - All QKV casts on DVE; ACT runs exp only (ACT is the second-busiest
    engine at ~81us of exp).  Proj evictions split DVE/ACT 3:1.
  - GpSimd keeps only the diagonal trimask mults.
PSUM: scores tag 2 banks x2 + pv 2 banks x1 + aux (qkv/proj) 1 bank x2
    = 8 banks exactly.
"""

import numpy as np

B, T, C = 2, 2048, 1024
H, D = 16, 64
NCORES = 8
HPC = H // NCORES          # heads per core = 2
BT = B * T                 # 4096 tokens total
TB = 512                   # token block (matmul moving free dim)
CK = C // 128              # 8 contraction tiles for the projections
NTB = BT // TB             # 8 token blocks
NQB = T // TB              # 4 q blocks per batch
NKT = T // 128             # 16 kt tiles per batch
SCALE = 1.0 / 32.0         # 1 / sqrt(C)


def build_program():
    """Build the single-core Bass program (same program runs on all 8 cores)."""
    from contextlib import ExitStack

    import concourse.mybir as mybir
    import concourse.tile as tile
    from concourse import bacc

    dt = mybir.dt
    F32 = dt.float32
    F16 = dt.float16

    nc = bacc.Bacc("TRN2")
    # all inputs pre-swizzled on the host into the exact SBUF layouts, so
    # every load is a contiguous-per-partition DMA at full ring rate (the
    # v3 strided w gather ran at ~80GB/s and pushed the first matmul to 17us)
    xH = nc.dram_tensor("xH", [NTB, 128, CK, TB], F16, kind="ExternalInput").ap()
    wqkvT = nc.dram_tensor("wqkvT", [128, 3, CK, 128], F16, kind="ExternalInput").ap()
    wpT = nc.dram_tensor("wpT", [HPC * D, C], F16, kind="ExternalInput").ap()
    # causal keep-mask: trimask[kt, qt] = 1.0 where kt <= qt
    trimask = nc.dram_tensor("trimask", [128, 128], F16, kind="ExternalInput").ap()
    y = nc.dram_tensor("y", [BT, C], F16, kind="ExternalOutput").ap()

    with ExitStack() as ctx:
        tc = ctx.enter_context(tile.TileContext(nc))
        const = ctx.enter_context(tc.tile_pool(name="const", bufs=1))
        xpool = ctx.enter_context(tc.tile_pool(name="xload", bufs=NTB))
        ppool = ctx.enter_context(tc.tile_pool(name="pexp", bufs=6))
        npool = ctx.enter_context(tc.tile_pool(name="norm", bufs=2))
        ypool = ctx.enter_context(tc.tile_pool(name="yout", bufs=4))
        psS = ctx.enter_context(tc.tile_pool(name="psS", bufs=2, space="PSUM"))
        psPV = ctx.enter_context(tc.tile_pool(name="psPV", bufs=1, space="PSUM"))
        psAux = ctx.enter_context(tc.tile_pool(name="psAux", bufs=2, space="PSUM"))

        # ---------- persistent SBUF ----------
        w_sb = const.tile([128, 3, CK, 128], F16, name="w_sb")
        wp_sb = const.tile([128, C], F16, name="wp_sb")
        trimask2 = const.tile([128, HPC, 128], F16, name="trimask2")

        # Per-batch transposed activations, heads packed on partitions
        # (h0 -> partitions 0:64, h1 -> 64:128).
        qT = [const.tile([128, T], F16, name=f"qT{b}") for b in range(B)]
        kT = [const.tile([128, T], F16, name=f"kT{b}") for b in range(B)]
        vT = [const.tile([128, T], F16, name=f"vT{b}") for b in range(B)]
        attnT = [const.tile([128, T], F16, name=f"attnT{b}") for b in range(B)]

        # [ones(64) | V(64)] stationary for PV: cols 0:64 = 1.0 so the PV
        # accumulation lands the softmax sums on PSUM partitions 0:64
        # (replicated 64x, so the custom-DVE reciprocal gets its required
        # partition-0 source with no copies); cols 64:128 = V natural
        # [kt, d], DMA-transposed in place.
        V2 = const.tile([128, B, HPC, NKT, 128], F16, name="V2")

        # ---------- x block loads (one DMA per token block) ----------
        xs = {}

        def issue_x(tb):
            xt = xpool.tile([128, CK, TB], F16, name=f"x{tb}", tag="x")
            nc.sync.dma_start(xt[:], xH[tb])
            xs[tb] = xt

        # ---------- QKV sub-steps (one fi section of one token block) ----------
        dest = {0: qT, 1: kT, 2: vT}

        def qkv_sub(tb, fi, cast="vector"):
            b, tcol = divmod(tb, NTB // B)
            ps = psAux.tile([128, TB], F32, name="qkv_ps", tag="aux")
            for ci in range(CK):
                nc.tensor.matmul(
                    ps[:],
                    w_sb[:, fi, ci, :],
                    xs[tb][:, ci, :],
                    start=(ci == 0),
                    stop=(ci == CK - 1),
                )
            out = dest[fi][b][:, tcol * TB : (tcol + 1) * TB]
            if cast == "scalar":
                nc.scalar.copy(out=out, in_=ps[:])
            else:
                nc.vector.tensor_copy(out=out, in_=ps[:])
            if fi == 2:
                # V natural tiles for this block: XBAR transpose on the DMA
                # engines straight into the [ones|V] stationary layout.
                for h in range(HPC):
                    hp = slice(h * 64, (h + 1) * 64)
                    nc.sync.dma_start(
                        V2[:, b, h, 4 * tcol : 4 * tcol + 4, 64:128],
                        vT[b][hp, tcol * TB : (tcol + 1) * TB],
                        transpose=True,
                    )
                # x prefetch cadence: one block per hook, emitted after the
                # transposes.  Tile hands DMA-completion semaphores out from
                # a small rotating pool and a reader's wait threshold counts
                # every increment emitted before it on that semaphore — so a
                # block's DMA must be emitted AFTER the readers of the block
                # ~8 DMAs earlier, or those readers stall on this transfer
                # (the v4 trace lost 9us exactly this way).
                t2 = tb + 2
                if t2 < NTB and t2 not in xs:
                    issue_x(t2)

        # ---------- phase 1: batch-0 QKV ----------
        # critical path first: the fi=2 weight section and x block 0 gate the
        # first matmul; everything else queues behind them on the DMA ring in
        # deadline order
        # deadline order on one ring (the ring ramps from ~60GB/s over the
        # first ~8us, and each HWDGE queue ramps separately — splitting the
        # critical path across queues just pays the cold ramp twice): the
        # fi=2 weight section first (block 0 computes V first), then x0 in
        # two halves so the first ci matmuls overlap the rest
        nc.sync.dma_start(w_sb[:, 2], wqkvT[:, 2])
        x0 = xpool.tile([128, CK, TB], F16, name="x0", tag="x")
        nc.sync.dma_start(x0[:, 0:4, :], xH[0, :, 0:4, :])
        nc.sync.dma_start(x0[:, 4:8, :], xH[0, :, 4:8, :])
        xs[0] = x0
        nc.sync.dma_start(w_sb[:, 0], wqkvT[:, 0])
        for _h in range(HPC):
            nc.sync.dma_start(trimask2[:, _h, :], trimask)
        nc.sync.dma_start(w_sb[:, 1], wqkvT[:, 1])
        issue_x(1)
        nc.sync.dma_start(wp_sb[:], wpT)
        nc.vector.memset(V2[:, :, :, :, 0:64], 1.0)

        # HAM pre-warm: the PE would otherwise sit idle from the end of the
        # framework preamble (~5.5us) until x0 lands (~13us), starting the
        # real QKV at 1.2GHz and re-throttling mid-wait.  ~24 throwaway
        # matmuls on scratch SBUF keep the PE busy through the DMA wait so
        # the real stream starts at the full 2.4GHz.
        wm_w = const.tile([128, 16], F16, name="wm_w")
        wm_x = const.tile([128, TB], F16, name="wm_x")
        nc.gpsimd.memset(wm_w[:], 0.0)
        nc.gpsimd.memset(wm_x[:], 0.0)
        wps = psPV.tile([128, HPC, TB], F32, name="warm_ps", tag="pv")
        for _i in range(12):
            nc.tensor.matmul(
                wps[0:16, 0, :], wm_w[:], wm_x[:], start=True, stop=True
            )

        # ---------- deferred work: batch-1 QKV, then projections ----------
        pending = []
        for tb in range(NTB // B, NTB):
            for fi in (2, 0, 1):  # V first, as above
                pending.append(
                    ("qkv", tb, lambda tb=tb, fi=fi, **kw: qkv_sub(tb, fi, **kw))
                )

        def emit_pending(n=None):
            # Inject up to n deferred sub-steps.  A qkv sub-step is only
            # eligible once its x block DMA has been issued (issue order
            # chains through the fi==2 hooks); ineligible items are skipped
            # so later, independent proj items can still fill the PE.
            cnt = len(pending) if n is None else n
            emitted = 0
            i = 0
            while emitted < cnt and i < len(pending):
                kind, tb, fn = pending[i]
                if kind == "qkv" and tb not in xs:
                    i += 1
                    continue
                pending.pop(i)[2]()
                emitted += 1

        def emit_pending_qkv():
            # in-order drain: block tb's v sub-step issues x for tb+2, so the
            # FIFO order itself guarantees xs is populated before use.
            # Alternate the casts over Scalar/Vector so the b1 attention
            # scores don't all queue behind one engine's cast backlog.
            n = 0
            while any(k == "qkv" for k, _, _ in pending):
                n += 1
                pending.pop(0)[2](cast="scalar" if n % 2 == 0 else "vector")

        nproj = [0]

        def proj_sub(b, ti, copy_engine):
            ysb = ypool.tile([128, 2, TB], F16, name="ysb", tag="y")
            rows = slice(b * T + ti * 128, b * T + (ti + 1) * 128)
            for fb in range(C // TB):
                ps = psAux.tile([128, TB], F32, name="y_ps", tag="aux")
                nc.tensor.matmul(
                    ps[:],
                    attnT[b][:, ti * 128 : (ti + 1) * 128],
                    wp_sb[:, fb * TB : (fb + 1) * TB],
                    start=True,
                    stop=True,
                )
                # evictions 2:1 DVE:ACT to balance ACT's exp load
                nproj[0] += 1
                if nproj[0] % 3 == 0:
                    nc.scalar.copy(out=ysb[:, fb, :], in_=ps[:])
                else:
                    nc.vector.tensor_copy(out=ysb[:, fb, :], in_=ps[:])
            nc.sync.dma_start(y[rows, :], ysb[:])

        # ---------- main loop: QKV blocks interleaved with attention ----------
        # b0's q-block i only needs token block i of Q and blocks <= i of
        # K/V, so each b0 block is emitted right before the q-block that
        # completes its dependencies -- no phase boundary, exp starts ~6us in.
        # V sections run one q-block AHEAD (block 0's in a tiny pre-phase) so
        # the v-cast -> DMA-transpose chain lands well before the diagonal PV
        # that needs it.
        qkv_sub(0, 2)
        for b in range(B):
            if b == 1:
                emit_pending_qkv()
            for qb in range(NQB):
                if b == 0:
                    for fi in (0, 1):
                        qkv_sub(qb, fi)
                    if qb + 1 < NQB:
                        qkv_sub(qb + 1, 2)
                nkt = 4 * qb + 4
                pv = psPV.tile([128, HPC, TB], F32, name="pv_ps", tag="pv")
                stages = []  # deferred PV matmuls, two kt tiles behind scores

                def flush(keep=0):
                    while len(stages) > keep:
                        stages.pop(0)()

                for kti in range(nkt):
                    qs = max(0, kti * 128 - qb * TB)  # local col start
                    N = TB - qs
                    sps = psS.tile([128, HPC, TB], F32, name="s_ps", tag="s")
                    for h in range(HPC):
                        hp = slice(h * 64, (h + 1) * 64)
                        nc.tensor.matmul(
                            sps[:, h, 0:N],
                            kT[b][hp, kti * 128 : (kti + 1) * 128],
                            qT[b][hp, qb * TB + qs : (qb + 1) * TB],
                            start=True,
                            stop=True,
                        )
                    P = ppool.tile([128, HPC, TB], F16, name="Pt", tag="P")
                    nc.scalar.activation(
                        P[:, :, 0:N],
                        sps[:, :, 0:N],
                        mybir.ActivationFunctionType.Exp,
                        scale=SCALE,
                    )
                    if kti * 128 >= qb * TB:
                        # diagonal tile: first 128 local cols hold the
                        # triangle; one GpSimd mult covers both heads
                        nc.gpsimd.tensor_mul(
                            P[:, :, 0:128], P[:, :, 0:128], trimask2[:]
                        )

                    def pv_step(kti=kti, qs=qs, N=N, P=P):
                        for h in range(HPC):
                            nc.tensor.matmul(
                                pv[:, h, qs:TB],
                                V2[:, b, h, kti, :],
                                P[:, h, 0:N],
                                start=(kti == 0),
                                stop=(kti == nkt - 1),
                            )

                    stages.append(pv_step)
                    flush(keep=3)
                    # in the last q-block drain pending unconditionally —
                    # anything left after it would run serialized (and HAM-
                    # cold) behind the tail
                    last = b == 1 and qb == NQB - 1
                    if kti % 2 == 1 and len(pending) > (0 if last else 8):
                        emit_pending(1)
                flush()
                # fill the PE while the normalization drains, so the next
                # q-block's first PV matmul doesn't head-of-line block on the
                # single-buffered pv PSUM tile.
                emit_pending(4)

                if b == 1 and qb == NQB - 1:
                    # drain tail: normalize per 128-token chunk and project
                    # it immediately, pipelining DVE normalize / PE matmul /
                    # ACT eviction so the tail after the last PV is short and
                    # the PE never idles long enough to re-throttle HAM.
                    # The scores pools are dead here, so the proj psum (both
                    # 512-col halves as one 2-bank tile) and the exp pool's
                    # ysb stand-ins come from them.
                    for ti in range(4 * qb, 4 * qb + 4):
                        lo = (ti - 4 * qb) * 128
                        rsc = npool.tile([64, HPC, 128], F32, name="rsc", tag="rsb")
                        nc.vector.reciprocal_approx_fast(
                            rsc[:], pv[0:64, :, lo : lo + 128]
                        )
                        for h in range(HPC):
                            hp = slice(h * 64, (h + 1) * 64)
                            nc.vector.tensor_mul(
                                attnT[b][hp, qb * TB + lo : qb * TB + lo + 128],
                                pv[64:128, h, lo : lo + 128],
                                rsc[:, h, :],
                            )
                        ysb = ypool.tile([128, 2, TB], F16, name="ysb", tag="y")
                        pps = psS.tile([128, HPC, TB], F32, name="yt_ps", tag="s")
                        for fb in range(C // TB):
                            nc.tensor.matmul(
                                pps[:, fb, :],
                                attnT[b][:, ti * 128 : (ti + 1) * 128],
                                wp_sb[:, fb * TB : (fb + 1) * TB],
                                start=True,
                                stop=True,
                            )
                        nc.scalar.copy(out=ysb[:], in_=pps[:])
                        rows = slice(b * T + ti * 128, b * T + (ti + 1) * 128)
                        nc.sync.dma_start(y[rows, :], ysb[:])
                else:
                    # normalize: sums sit replicated on PSUM partitions 0:64
                    # (ones-first), so one partition-0-sourced reciprocal
                    # covers both heads; one DVE mult per head writes attnT.
                    rsb = npool.tile([64, HPC, TB], F32, name="rsb", tag="rsb")
                    nc.vector.reciprocal_approx_fast(rsb[:], pv[0:64, :, :])
                    for h in range(HPC):
                        hp = slice(h * 64, (h + 1) * 64)
                        nc.vector.tensor_mul(
                            attnT[b][hp, qb * TB : (qb + 1) * TB],
                            pv[64:128, h, :],
                            rsb[:, h, :],
                        )
                    for ti in range(4 * qb, 4 * qb + 4):
                        pending.append(
                            ("proj", -1, lambda b=b, ti=ti: proj_sub(b, ti, "mix"))
                        )
        emit_pending()
    nc.compile()
    return nc


def make_in_maps(x, w_attn, w_proj):
    """Host-side sharding into the per-core layouts."""
    x = np.asarray(x, dtype=np.float32)
    w_attn = np.asarray(w_attn, dtype=np.float32)
    w_proj = np.asarray(w_proj, dtype=np.float32)

    # x pre-swizzled block-major: xH[tb, p, ci, j] = x_flat[tb*TB + j, ci*128 + p]
    xT = x.reshape(BT, C).T.astype(np.float16)              # [C, BT]
    xH = np.ascontiguousarray(
        xT.reshape(CK, 128, NTB, TB).transpose(2, 1, 0, 3)  # [NTB, 128, CK, TB]
    )
    wpT_full = np.ascontiguousarray(w_proj.T.astype(np.float16))
    trimask = np.ascontiguousarray(
        np.tril(np.ones((128, 128), np.float16)).T  # keep kt <= qt
    )

    in_maps = []
    for c in range(NCORES):
        rows = []
        for sec in range(3):                                # q, k, v
            for h in (HPC * c, HPC * c + 1):
                rows.extend(range(sec * C + h * D, sec * C + (h + 1) * D))
        wq = w_attn[rows, :].T.astype(np.float16)           # [C, 3*HPC*D]
        # fi-major, matching the w_sb SBUF layout [128, 3, CK, 128]
        wqkvT = np.ascontiguousarray(
            wq.reshape(CK, 128, 3, 128).transpose(1, 2, 0, 3)
        )
        wpT = np.ascontiguousarray(
            wpT_full[c * HPC * D : (c + 1) * HPC * D, :]    # [128, 1024]
        )
        in_maps.append({"xH": xH, "wqkvT": wqkvT, "wpT": wpT, "trimask": trimask})
    return in_maps


_PROGRAM = None


def _program():
    global _PROGRAM
    if _PROGRAM is None:
        _PROGRAM = build_program()
    return _PROGRAM


def kernel(x, w_attn, w_proj):
    from concourse.bass_utils import run_bass_kernel_spmd

    res = run_bass_kernel_spmd(
        _program(), make_in_maps(x, w_attn, w_proj), list(range(NCORES))
    )
    out = res.results[0]["y"].astype(np.float32, copy=True)
    for i in range(1, NCORES):
        out += res.results[i]["y"]
    return out.reshape(B, T, C)
